# revision 24
# baseline (speedup 1.0000x reference)
"""KAN-GNN message passing on 8 TRN2 NeuronCores.

Strategy (data-parallel over nodes, per sharding hint):
 - Nodes are assigned to cores by a balanced 4-coloring: color c maps to the
   core pair {2c, 2c+1} and therefore to a contiguous 25088-row block of the
   AllGathered tables.  The coloring greedily splits every target's source
   list evenly across the 4 blocks, so the per-block slot rectangles stay
   near ceil(deg/4).  Within a color, nodes are dealt to its two cores by
   in-degree round-robin, keeping the per-core degree profiles matched.
 - Per core: KAN layer 1 on its node shard (3 fused matmuls per group of
   128 nodes; 4 groups share one PSUM bank so bias-add + relu run batched),
   then AllGather of the bf16 h1 table.
 - Aggregation: groups are packed into batches; per batch, 4 dma_gather
   calls (one per 25088-row table block, int16 indices relative to the
   block) pull all (target, slot) source rows into SBUF side by side; a
   halving tree of strided DVE adds reduces each block's slots for all
   groups at once, 3 adds combine the block partials, scale by 1/deg,
   PE-transpose, KAN layer 2, AllGather of the f32 h2 table (f32 because
   dma_gather rows must be a multiple of 256B), second batched
   gather/reduce, then a batched log_softmax (Exp/Ln tables load once).
 - All indices/permutations are precomputed on the host and baked into the
   (single, SPMD) program; per-core data goes in as input tensors.
"""
import numpy as np
import ml_dtypes

import concourse.bacc as bacc
import concourse.mybir as mybir
import concourse.tile as tile
import concourse.bass as bass
from concourse.bass_utils import run_bass_kernel_spmd

N_NODES = 100000
N_EDGES = 1600000
IN_F, HID_F, OUT_F = 128, 128, 64
K = 8               # cores
P = 128             # partitions / targets per group
J = 12544           # local nodes per core (98 * 128), 12500 real + 44 pad
G = J // P          # 98 groups
JREAL = N_NODES // K  # 12500
TBL = K * J         # 100352 rows in the all-gathered tables
NC_COL = 4          # colors = table blocks = core pairs
BLK = 2 * J         # 25088 rows per block (int16-addressable)
CREAL = 2 * JREAL   # real nodes per color
PAD_REL = JREAL     # block-relative all-zero row (first core's pad area)

RCAP = 48           # max padded slots (ng * D_R) per range-block per batch
NGMAX = 8           # max groups per batch
NB1 = 4             # phase-1 groups per PSUM bank block

BF16 = mybir.dt.bfloat16
F32 = mybir.dt.float32
I16 = mybir.dt.int16


def _color_sources(src, tgt, deg):
    """Greedy balanced 4-coloring of nodes: minimize per-target color skew.

    Returns colors[n] in 0..3 with exactly CREAL nodes per color.
    """
    eorder = np.argsort(src, kind="stable")
    tgt_by_src = tgt[eorder]
    indptr = np.zeros(N_NODES + 1, dtype=np.int64)
    np.cumsum(np.bincount(src, minlength=N_NODES), out=indptr[1:])

    cnt = np.zeros((N_NODES, NC_COL), dtype=np.int32)
    quota = np.ceil(np.maximum(deg, 1) / NC_COL).astype(np.int32)
    colors = np.full(N_NODES, -1, dtype=np.int8)
    sizes = np.zeros(NC_COL, dtype=np.int64)
    # process high-out-degree nodes first
    odeg = indptr[1:] - indptr[:-1]
    proc = np.argsort(-odeg, kind="stable")
    big = np.int64(1) << 40
    for n in proc:
        t = tgt_by_src[indptr[n]:indptr[n + 1]]
        ct = cnt[t, :]
        # hard penalty for pushing any target past its per-color quota
        cost = (ct >= quota[t][:, None]).sum(axis=0) * 10000 + ct.sum(axis=0)
        cost = cost + np.where(sizes >= CREAL, big, 0)
        c = int(np.argmin(cost))
        colors[n] = c
        sizes[c] += 1
        np.add.at(cnt[:, c], t, 1)
    # refinement passes: re-greedy each node with warm counts (size-neutral
    # moves only happen when strictly better, tracking exact sizes)
    for _ in range(2):
        for n in proc:
            t = tgt_by_src[indptr[n]:indptr[n + 1]]
            c0 = colors[n]
            np.add.at(cnt[:, c0], t, -1)
            sizes[c0] -= 1
            ct = cnt[t, :]
            cost = ((ct >= quota[t][:, None]).sum(axis=0) * 10000
                    + (ct >= (quota[t][:, None] + 1)).sum(axis=0) * 100000
                    + ct.sum(axis=0))
            cost = cost + np.where(sizes >= CREAL, big, 0)
            c = int(np.argmin(cost))
            colors[n] = c
            sizes[c] += 1
            np.add.at(cnt[:, c], t, 1)
    return colors


def _host_prep(x, edge_index, w1, b1, c1, w2, b2, c2):
    src = np.asarray(edge_index[0], dtype=np.int64)
    tgt = np.asarray(edge_index[1], dtype=np.int64)
    x = np.asarray(x, dtype=np.float32)

    deg = np.bincount(tgt, minlength=N_NODES)
    colors = _color_sources(src, tgt, deg)

    # within each color: in-degree-sorted round-robin onto its two cores
    core_of_node = np.empty(N_NODES, dtype=np.int64)
    j_of_node = np.empty(N_NODES, dtype=np.int64)
    for c in range(NC_COL):
        nodes_c = np.where(colors == c)[0]
        nodes_c = nodes_c[np.argsort(-deg[nodes_c], kind="stable")]
        core_of_node[nodes_c] = 2 * c + (np.arange(len(nodes_c)) % 2)
        j_of_node[nodes_c] = np.arange(len(nodes_c)) // 2
    pos_of_node = core_of_node * J + j_of_node

    degs_kj = np.zeros((K, J), dtype=np.int64)
    degs_kj[core_of_node, j_of_node] = deg

    # per-(group, color) slot rectangle heights, shared across cores
    ek = core_of_node[tgt]
    ej = j_of_node[tgt]
    ecol = colors[src].astype(np.int64)
    gs_all = ej // P
    cnt_gc = np.zeros((K, G, P, NC_COL), dtype=np.int64)
    np.add.at(cnt_gc, (ek, gs_all, ej % P, ecol), 1)
    Dgr = np.maximum(cnt_gc.max(axis=(0, 2)), 1)      # [G, NC_COL]

    # pack consecutive groups into batches with common per-color heights
    batches = []        # dicts: ga, ng, Ds[4], slot_off (gt cols), idx_off
    icol_off = 0
    g = 0
    while g < G:
        ng = 1
        Ds = Dgr[g].copy()
        while g + ng < G and ng < NGMAX:
            nD = np.maximum(Ds, Dgr[g + ng])
            if (ng + 1) * int(nD.max()) > RCAP:
                break
            Ds = nD
            ng += 1
        batches.append({
            "ga": g, "ng": ng, "Ds": [int(d) for d in Ds],
            "icol": icol_off,
        })
        icol_off += 8 * ng * int(Ds.sum())   # int16 cols (= rows/16)
        g += ng
    TOTC = icol_off

    # index tensor [K][128, TOTC] int16 (wrap-16, replicated across 8 bands)
    idx_all = np.zeros((K, P, TOTC), dtype=np.int16)
    # edge slot positions: sort edges by (core, target j, color) to rank them
    key = ((ek * J) + ej) * NC_COL + ecol
    eorder = np.argsort(key, kind="stable")
    skey = key[eorder]
    _, counts = np.unique(skey, return_counts=True)
    run_starts = np.concatenate([[0], np.cumsum(counts)[:-1]])
    d_in_run = np.arange(len(skey)) - np.repeat(run_starts, counts)
    eks = skey // (J * NC_COL)
    ejs = (skey // NC_COL) % J
    ecs = skey % NC_COL
    egs = ejs // P
    eps = ejs % P
    rel = (pos_of_node[src[eorder]] - ecs * BLK).astype(np.int64)
    assert (rel >= 0).all() and (rel < BLK).all()

    ga_of_g = np.zeros(G, dtype=np.int64)
    for bi, bt in enumerate(batches):
        for gg in range(bt["ng"]):
            ga_of_g[bt["ga"] + gg] = bi
    # flat list position within the batch's idx block for each edge:
    # ranges laid out color-major: color R block has ng*Ds[R] columns of 128.
    bts = batches
    b_of_e = ga_of_g[egs]
    icol_b = np.array([bt["icol"] for bt in bts], dtype=np.int64)
    ng_b = np.array([bt["ng"] for bt in bts], dtype=np.int64)
    ga_b = np.array([bt["ga"] for bt in bts], dtype=np.int64)
    Ds_b = np.array([bt["Ds"] for bt in bts], dtype=np.int64)       # [B, 4]
    DsCum_b = np.concatenate([np.zeros((len(bts), 1), np.int64),
                              np.cumsum(Ds_b, axis=1)], axis=1)     # [B, 5]
    # column within gather dest for this edge's slot (color-major blocks):
    gg_e = egs - ga_b[b_of_e]
    cc = ng_b[b_of_e] * DsCum_b[b_of_e, ecs] + gg_e * Ds_b[b_of_e, ecs] + d_in_run
    jlist = cc * P + eps                   # position in the batch's flat list
    # band 0 (partitions 0-15): wrap-16 layout; pads -> PAD_REL (zero row)
    col16 = icol_b[b_of_e] + jlist // 16
    row16 = jlist % 16
    idx_all[:, :16, :] = PAD_REL
    idx_all[eks, row16, col16] = rel.astype(np.int16)
    # bands 1..7 are exact replicas of band 0
    for band in range(1, 8):
        idx_all[:, 16 * band:16 * (band + 1), :] = idx_all[:, :16, :]

    # per-core 1/deg  [K, P, G]  (0 for pad targets)
    with np.errstate(divide="ignore"):
        dr = 1.0 / np.maximum(degs_kj, 1).astype(np.float32)
    real = np.zeros((K, J), dtype=np.float32)
    real[:, :JREAL] = 1.0
    degrecip = (dr * np.where(real > 0, 1.0, 0.0)).reshape(K, G, P).transpose(0, 2, 1).copy()

    # pad-node mask [P, G] (same on every core)
    mask_j = (np.arange(J) < JREAL).astype(np.float32)
    mask_pg = mask_j.reshape(G, P).T.copy()

    # xT shards, bf16 [K][IN_F, J]
    xT = np.zeros((K, IN_F, J), dtype=ml_dtypes.bfloat16)
    for k in range(K):
        nodes_k = np.where(core_of_node == k)[0]
        xT[k][:, j_of_node[nodes_k]] = x[nodes_k].T.astype(ml_dtypes.bfloat16)

    # fused KAN weights
    A1 = (w1 + 0.1 * c1[:, :, 0]).astype(ml_dtypes.bfloat16)
    B1 = (0.1 * c1[:, :, 1]).astype(ml_dtypes.bfloat16)
    C1 = (0.1 * c1[:, :, 2]).astype(ml_dtypes.bfloat16)
    A2 = (w2 + 0.1 * c2[:, :, 0]).astype(ml_dtypes.bfloat16)
    B2 = (0.1 * c2[:, :, 1]).astype(ml_dtypes.bfloat16)
    C2 = (0.1 * c2[:, :, 2]).astype(ml_dtypes.bfloat16)
    b1b = np.tile(np.asarray(b1, np.float32)[None, :], (P, 1))
    b2b = np.tile(np.asarray(b2, np.float32)[None, :], (P, 1))
    ident = np.eye(P, dtype=np.float32)

    in_maps = []
    for k in range(K):
        in_maps.append({
            "xT": xT[k],
            "idx": idx_all[k],
            "degrecip": degrecip[k],
            "mask": mask_pg,
            "A1": A1, "B1": B1, "C1": C1,
            "A2": A2, "B2": B2, "C2": C2,
            "b1b": b1b, "b2b": b2b, "ident": ident,
        })
    meta = {"batches": batches, "TOTC": TOTC,
            "core_of_node": core_of_node, "j_of_node": j_of_node}
    return in_maps, meta


def build_program(batches, TOTC, dump=False):
    nc = bacc.Bacc("TRN2", target_bir_lowering=False, debug=False, num_devices=K,
                   dynamic_dma_scratch_size=32768, num_swdge_queues=4)

    xT = nc.dram_tensor("xT", [IN_F, J], BF16, kind="ExternalInput")
    idx = nc.dram_tensor("idx", [P, TOTC], I16, kind="ExternalInput")
    degrecip = nc.dram_tensor("degrecip", [P, G], F32, kind="ExternalInput")
    mask = nc.dram_tensor("mask", [P, G], F32, kind="ExternalInput")
    A1 = nc.dram_tensor("A1", [IN_F, HID_F], BF16, kind="ExternalInput")
    B1 = nc.dram_tensor("B1", [IN_F, HID_F], BF16, kind="ExternalInput")
    C1 = nc.dram_tensor("C1", [IN_F, HID_F], BF16, kind="ExternalInput")
    A2 = nc.dram_tensor("A2", [HID_F, OUT_F], BF16, kind="ExternalInput")
    B2 = nc.dram_tensor("B2", [HID_F, OUT_F], BF16, kind="ExternalInput")
    C2 = nc.dram_tensor("C2", [HID_F, OUT_F], BF16, kind="ExternalInput")
    b1b = nc.dram_tensor("b1b", [P, HID_F], F32, kind="ExternalInput")
    b2b = nc.dram_tensor("b2b", [P, OUT_F], F32, kind="ExternalInput")
    ident = nc.dram_tensor("ident", [P, P], F32, kind="ExternalInput")
    y = nc.dram_tensor("y", [J, OUT_F], F32, kind="ExternalOutput")
    if dump:
        h1o = nc.dram_tensor("h1o", [J, HID_F], BF16, kind="ExternalOutput")
        sno = nc.dram_tensor("sno", [J, HID_F], F32, kind="ExternalOutput")

    h1_in = nc.dram_tensor("h1_in", [J, HID_F], BF16, kind="Internal")
    h1_tbl = nc.dram_tensor("h1_tbl", [TBL, HID_F], BF16, kind="Internal",
                            addr_space="Shared")
    h2_in = nc.dram_tensor("h2_in", [J, OUT_F], F32, kind="Internal")
    h2_tbl = nc.dram_tensor("h2_tbl", [TBL, OUT_F], F32, kind="Internal",
                            addr_space="Shared")

    with tile.TileContext(nc) as tc:
        with (
            tc.tile_pool(name="consts", bufs=1) as cpool,
            tc.tile_pool(name="p1", bufs=2) as p1pool,
            tc.tile_pool(name="work", bufs=2) as wpool,
            tc.tile_pool(name="g1p", bufs=2) as g1pool,
            tc.tile_pool(name="idxp", bufs=2) as ipool,
            tc.tile_pool(name="psum", bufs=2, space="PSUM") as ppool,
        ):
            c_dr = cpool.tile([P, G], F32, tag="dr")
            nc.sync.dma_start(out=c_dr[:], in_=degrecip[:, :])
            c_mask = cpool.tile([P, G], F32, tag="mask")
            nc.sync.dma_start(out=c_mask[:], in_=mask[:, :])
            c_w1 = []
            for nm, t in (("A1", A1), ("B1", B1), ("C1", C1)):
                w = cpool.tile([IN_F, HID_F], BF16, tag=nm)
                nc.sync.dma_start(out=w[:], in_=t[:, :])
                c_w1.append(w)
            c_w2 = []
            for nm, t in (("A2", A2), ("B2", B2), ("C2", C2)):
                w = cpool.tile([HID_F, OUT_F], BF16, tag=nm)
                nc.sync.dma_start(out=w[:], in_=t[:, :])
                c_w2.append(w)
            c_b1 = cpool.tile([P, HID_F], F32, tag="b1b")
            nc.sync.dma_start(out=c_b1[:], in_=b1b[:, :])
            c_b2 = cpool.tile([P, OUT_F], F32, tag="b2b")
            nc.sync.dma_start(out=c_b2[:], in_=b2b[:, :])
            c_id = cpool.tile([P, P], F32, tag="ident")
            nc.sync.dma_start(out=c_id[:], in_=ident[:, :])

            # ---------------- phase 1: KAN layer 1 on the shard ----------------
            sc_p1, _ = nc.enter_named_scope("phase1", False)
            blocks = [(b * NB1, NB1) for b in range(96 // NB1)] + [(96, 1), (97, 1)]
            for (ga, nb) in blocks:
                w = nb * P
                xt = p1pool.tile([IN_F, NB1 * P], BF16, tag="xt")
                nc.sync.dma_start(out=xt[:, :w], in_=xT[:, ga * P:ga * P + w])
                x2 = p1pool.tile([IN_F, NB1 * P], BF16, tag="x2")
                nc.vector.tensor_tensor(out=x2[:, :w], in0=xt[:, :w], in1=xt[:, :w],
                                        op=mybir.AluOpType.mult)
                x3 = p1pool.tile([IN_F, NB1 * P], BF16, tag="x3")
                nc.vector.tensor_tensor(out=x3[:, :w], in0=x2[:, :w], in1=xt[:, :w],
                                        op=mybir.AluOpType.mult)
                ps = ppool.tile([P, NB1 * HID_F], F32, tag="k1")
                for gg in range(nb):
                    sl = slice(gg * P, (gg + 1) * P)
                    ol = slice(gg * HID_F, (gg + 1) * HID_F)
                    nc.tensor.matmul(out=ps[:, ol], lhsT=xt[:, sl], rhs=c_w1[0][:],
                                     start=True, stop=False)
                    nc.tensor.matmul(out=ps[:, ol], lhsT=x2[:, sl], rhs=c_w1[1][:],
                                     start=False, stop=False)
                    nc.tensor.matmul(out=ps[:, ol], lhsT=x3[:, sl], rhs=c_w1[2][:],
                                     start=False, stop=True)
                hb = p1pool.tile([P, NB1 * HID_F], F32, tag="hb")
                b1bc = c_b1[:, :].unsqueeze(1).to_broadcast([P, nb, HID_F])
                nc.vector.tensor_tensor(
                    out=hb[:, :nb * HID_F].rearrange("p (g f) -> p g f", g=nb),
                    in0=ps[:, :nb * HID_F].rearrange("p (g f) -> p g f", g=nb),
                    in1=b1bc, op=mybir.AluOpType.add)
                h1t = p1pool.tile([P, NB1 * HID_F], BF16, tag="h1t")
                if ga == 97:
                    nc.scalar.activation(out=h1t[:, :HID_F], in_=hb[:, :HID_F],
                                         func=mybir.ActivationFunctionType.Relu,
                                         scale=c_mask[:, 97:98])
                else:
                    nc.scalar.activation(out=h1t[:, :nb * HID_F],
                                         in_=hb[:, :nb * HID_F],
                                         func=mybir.ActivationFunctionType.Relu)
                nc.sync.dma_start(
                    out=h1_in[ga * P:(ga + nb) * P, :].rearrange(
                        "(g p) f -> p g f", p=P),
                    in_=h1t[:, :nb * HID_F].rearrange("p (g f) -> p g f", g=nb))
                if dump:
                    nc.sync.dma_start(
                        out=h1o[ga * P:(ga + nb) * P, :].rearrange(
                            "(g p) f -> p g f", p=P),
                        in_=h1t[:, :nb * HID_F].rearrange("p (g f) -> p g f", g=nb))
            nc.leave_named_scope("phase1", sc_p1, False)

            # ---------------- AllGather h1 ----------------
            sc_ag1, _ = nc.enter_named_scope("ag1", False)
            nc.gpsimd.collective_compute(
                "AllGather", mybir.AluOpType.bypass,
                replica_groups=[list(range(K))],
                ins=[h1_in[:, :]], outs=[h1_tbl[:, :]],
            )
            nc.leave_named_scope("ag1", sc_ag1, False)

            def gather_reduce(bt, table, F):
                """4 block gathers + strided tree reduce; returns [P, ng, F]
                view of the combined partial sums (in the color-0 block)."""
                ga, ng, Ds = bt["ga"], bt["ng"], bt["Ds"]
                dsum = sum(Ds)
                it = ipool.tile([P, 8 * 4 * RCAP], I16, tag="idx")
                ncols16 = 8 * ng * dsum
                nc.sync.dma_start(out=it[:, :ncols16],
                                  in_=idx[:, bt["icol"]:bt["icol"] + ncols16])
                # separate byte buffer per range block so the 4 queue-pairs
                # run concurrently and batch n+1 gathers overlap batch n
                # reduces; shared between phases via bitcast.
                views = []
                for R in range(NC_COL):
                    D = Ds[R]
                    nidx = P * ng * D
                    gt_raw = g1pool.tile([P, RCAP * HID_F], BF16, tag=f"g{R}")
                    gt = gt_raw[:, :] if table.dtype == BF16 else \
                        gt_raw[:, :].bitcast(F32)
                    nc.gpsimd.dma_gather(
                        out_ap=gt[:, :ng * D * F].rearrange(
                            "p (c f) -> p c f", c=ng * D),
                        in_ap=table[R * BLK:(R + 1) * BLK, :],
                        idxs_ap=it[:, 8 * ng * sum(Ds[:R]):
                                   8 * ng * sum(Ds[:R]) + nidx // 16],
                        num_idxs=nidx,
                        num_idxs_reg=nidx,
                        elem_size=F,
                        single_packet=False,
                        queue_num=R,
                    )
                    v = gt[:, :ng * D * F].rearrange("p (g x) -> p g x", g=ng)
                    cur = D
                    while cur > 1:
                        h = cur // 2
                        nc.vector.tensor_tensor(
                            out=v[:, :, :h * F], in0=v[:, :, :h * F],
                            in1=v[:, :, (cur - h) * F:cur * F],
                            op=mybir.AluOpType.add)
                        cur = cur - h
                    views.append(v[:, :, :F])
                acc = views[0]
                for R in range(1, NC_COL):
                    nc.vector.tensor_tensor(out=acc, in0=acc, in1=views[R],
                                            op=mybir.AluOpType.add)
                return acc

            # ---------------- phase 2: aggregate + KAN layer 2 ----------------
            sc_p2, _ = nc.enter_named_scope("phase2", False)
            for bt in batches:
                ga, ng = bt["ga"], bt["ng"]
                acc = gather_reduce(bt, h1_tbl, HID_F)
                sn = wpool.tile([P, NGMAX * HID_F], F32, tag="sn")
                drb = c_dr[:, ga:ga + ng].unsqueeze(2).to_broadcast([P, ng, HID_F])
                nc.vector.tensor_tensor(
                    out=sn[:, :ng * HID_F].rearrange("p (g f) -> p g f", g=ng),
                    in0=acc, in1=drb, op=mybir.AluOpType.mult)
                if dump:
                    nc.sync.dma_start(
                        out=sno[ga * P:(ga + ng) * P, :].rearrange(
                            "(g p) f -> p g f", p=P),
                        in_=sn[:, :ng * HID_F].rearrange("p (g f) -> p g f", g=ng))
                for gg in range(ng):
                    g = ga + gg
                    pt = ppool.tile([P, P], F32, tag="tr")
                    nc.tensor.transpose(
                        out=pt[:], in_=sn[:, gg * HID_F:(gg + 1) * HID_F],
                        identity=c_id[:])
                    hT = wpool.tile([HID_F, P], BF16, tag="hT")
                    nc.scalar.copy(out=hT[:], in_=pt[:])
                    q2 = wpool.tile([HID_F, P], BF16, tag="q2")
                    nc.vector.tensor_tensor(out=q2[:], in0=hT[:], in1=hT[:],
                                            op=mybir.AluOpType.mult)
                    q3 = wpool.tile([HID_F, P], BF16, tag="q3")
                    nc.vector.tensor_tensor(out=q3[:], in0=q2[:], in1=hT[:],
                                            op=mybir.AluOpType.mult)
                    ps2 = ppool.tile([P, OUT_F], F32, tag="k2")
                    nc.tensor.matmul(out=ps2[:], lhsT=hT[:], rhs=c_w2[0][:],
                                     start=True, stop=False)
                    nc.tensor.matmul(out=ps2[:], lhsT=q2[:], rhs=c_w2[1][:],
                                     start=False, stop=False)
                    nc.tensor.matmul(out=ps2[:], lhsT=q3[:], rhs=c_w2[2][:],
                                     start=False, stop=True)
                    hb2 = wpool.tile([P, OUT_F], F32, tag="hb2")
                    nc.vector.tensor_tensor(out=hb2[:], in0=ps2[:], in1=c_b2[:],
                                            op=mybir.AluOpType.add)
                    h2t = wpool.tile([P, OUT_F], F32, tag="h2t")
                    nc.scalar.activation(out=h2t[:], in_=hb2[:],
                                         func=mybir.ActivationFunctionType.Copy,
                                         scale=c_mask[:, g:g + 1])
                    nc.sync.dma_start(out=h2_in[g * P:(g + 1) * P, :], in_=h2t[:])
            nc.leave_named_scope("phase2", sc_p2, False)

            # ---------------- AllGather h2 ----------------
            sc_ag2, _ = nc.enter_named_scope("ag2", False)
            nc.gpsimd.collective_compute(
                "AllGather", mybir.AluOpType.bypass,
                replica_groups=[list(range(K))],
                ins=[h2_in[:, :]], outs=[h2_tbl[:, :]],
            )
            nc.leave_named_scope("ag2", sc_ag2, False)

            # ---------------- phase 3: aggregate + log_softmax ----------------
            sc_p3, _ = nc.enter_named_scope("phase3", False)
            tn_all = cpool.tile([P, G * OUT_F], F32, tag="tn_all")
            se_all = cpool.tile([P, G], F32, tag="se_all")
            for bt in batches:
                ga, ng = bt["ga"], bt["ng"]
                acc = gather_reduce(bt, h2_tbl, OUT_F)
                tnv = tn_all[:, ga * OUT_F:(ga + ng) * OUT_F].rearrange(
                    "p (g f) -> p g f", g=ng)
                drb = c_dr[:, ga:ga + ng].unsqueeze(2).to_broadcast([P, ng, OUT_F])
                nc.vector.tensor_tensor(out=tnv, in0=acc, in1=drb,
                                        op=mybir.AluOpType.mult)
                mxb = wpool.tile([P, NGMAX], F32, tag="mxb")
                nc.vector.tensor_reduce(out=mxb[:, :ng].unsqueeze(2), in_=tnv,
                                        axis=mybir.AxisListType.X,
                                        op=mybir.AluOpType.max)
                nc.vector.tensor_tensor(
                    out=tnv, in0=tnv,
                    in1=mxb[:, :ng].unsqueeze(2).to_broadcast([P, ng, OUT_F]),
                    op=mybir.AluOpType.subtract)
                et = wpool.tile([P, NGMAX * OUT_F], F32, tag="et")
                nc.scalar.activation(
                    out=et[:, :ng * OUT_F],
                    in_=tn_all[:, ga * OUT_F:(ga + ng) * OUT_F],
                    func=mybir.ActivationFunctionType.Exp)
                nc.vector.tensor_reduce(
                    out=se_all[:, ga:ga + ng].unsqueeze(2),
                    in_=et[:, :ng * OUT_F].rearrange("p (g f) -> p g f", g=ng),
                    axis=mybir.AxisListType.X, op=mybir.AluOpType.add)
            lse = cpool.tile([P, G], F32, tag="lse")
            nc.scalar.activation(out=lse[:], in_=se_all[:],
                                 func=mybir.ActivationFunctionType.Ln)
            for bt in batches:
                ga, ng = bt["ga"], bt["ng"]
                ot = wpool.tile([P, NGMAX * OUT_F], F32, tag="ot")
                nc.vector.tensor_tensor(
                    out=ot[:, :ng * OUT_F].rearrange("p (g f) -> p g f", g=ng),
                    in0=tn_all[:, ga * OUT_F:(ga + ng) * OUT_F].rearrange(
                        "p (g f) -> p g f", g=ng),
                    in1=lse[:, ga:ga + ng].unsqueeze(2).to_broadcast(
                        [P, ng, OUT_F]),
                    op=mybir.AluOpType.subtract)
                nc.sync.dma_start(
                    out=y[ga * P:(ga + ng) * P, :].rearrange(
                        "(g p) f -> p g f", p=P),
                    in_=ot[:, :ng * OUT_F].rearrange("p (g f) -> p g f", g=ng))
            nc.leave_named_scope("phase3", sc_p3, False)

    nc.compile()
    return nc


def kernel(x, edge_index, w1, b1, c1, w2, b2, c2):
    in_maps, meta = _host_prep(x, edge_index, w1, b1, c1, w2, b2, c2)
    nc = build_program(meta["batches"], meta["TOTC"])
    res = run_bass_kernel_spmd(nc, in_maps, core_ids=list(range(K)))
    out = np.empty((N_NODES, OUT_F), dtype=np.float32)
    core_of, j_of = meta["core_of_node"], meta["j_of_node"]
    for k in range(K):
        nodes_k = np.where(core_of == k)[0]
        out[nodes_k] = res.results[k]["y"][j_of[nodes_k]]
    return out


# revision 28
# speedup vs baseline: 1.0422x; 1.0422x over previous
"""KAN-GNN message passing on 8 TRN2 NeuronCores.

Strategy (data-parallel over nodes, per sharding hint):
 - Nodes are assigned to cores by a balanced 4-coloring: color c maps to the
   core pair {2c, 2c+1} and therefore to a contiguous 25088-row block of the
   AllGathered tables.  The coloring greedily splits every target's source
   list evenly across the 4 blocks, so the per-block slot rectangles stay
   near ceil(deg/4).  Within a color, nodes are dealt to its two cores by
   in-degree round-robin, keeping the per-core degree profiles matched.
 - Per core: KAN layer 1 on its node shard (3 fused matmuls per group of
   128 nodes; 4 groups share one PSUM bank so bias-add + relu run batched),
   then AllGather of the bf16 h1 table.
 - Aggregation: groups are packed into batches; per batch, 4 dma_gather
   calls (one per 25088-row table block, int16 indices relative to the
   block) pull all (target, slot) source rows into SBUF side by side; a
   halving tree of strided DVE adds reduces each block's slots for all
   groups at once, 3 adds combine the block partials, scale by 1/deg,
   PE-transpose, KAN layer 2, AllGather of the f32 h2 table (f32 because
   dma_gather rows must be a multiple of 256B), second batched
   gather/reduce, then a batched log_softmax (Exp/Ln tables load once).
 - All indices/permutations are precomputed on the host and baked into the
   (single, SPMD) program; per-core data goes in as input tensors.
"""
import numpy as np
import ml_dtypes

import concourse.bacc as bacc
import concourse.mybir as mybir
import concourse.tile as tile
import concourse.bass as bass
from concourse.bass_utils import run_bass_kernel_spmd

N_NODES = 100000
N_EDGES = 1600000
IN_F, HID_F, OUT_F = 128, 128, 64
K = 8               # cores
P = 128             # partitions / targets per group
J = 12544           # local nodes per core (98 * 128), 12500 real + 44 pad
G = J // P          # 98 groups
JREAL = N_NODES // K  # 12500
TBL = K * J         # 100352 rows in the all-gathered tables
NC_COL = 4          # colors = table blocks = core pairs
BLK = 2 * J         # 25088 rows per block (int16-addressable)
CREAL = 2 * JREAL   # real nodes per color
PAD_REL = JREAL     # block-relative all-zero row (first core's pad area)

RCAP = 48           # max padded slots (ng * D_R) per range-block per batch
NGMAX = 8           # max groups per batch
NB1 = 4             # phase-1 groups per PSUM bank block

BF16 = mybir.dt.bfloat16
F32 = mybir.dt.float32
I16 = mybir.dt.int16


def _color_sources(src, tgt, deg):
    """Greedy balanced 4-coloring of nodes: minimize per-target color skew.

    Returns colors[n] in 0..3 with exactly CREAL nodes per color.
    """
    eorder = np.argsort(src, kind="stable")
    tgt_by_src = tgt[eorder]
    indptr = np.zeros(N_NODES + 1, dtype=np.int64)
    np.cumsum(np.bincount(src, minlength=N_NODES), out=indptr[1:])

    cnt = np.zeros((N_NODES, NC_COL), dtype=np.int32)
    quota = np.ceil(np.maximum(deg, 1) / NC_COL).astype(np.int32)
    colors = np.full(N_NODES, -1, dtype=np.int8)
    sizes = np.zeros(NC_COL, dtype=np.int64)
    # process high-out-degree nodes first
    odeg = indptr[1:] - indptr[:-1]
    proc = np.argsort(-odeg, kind="stable")
    big = np.int64(1) << 40
    for n in proc:
        t = tgt_by_src[indptr[n]:indptr[n + 1]]
        ct = cnt[t, :]
        # hard penalty for pushing any target past its per-color quota
        cost = (ct >= quota[t][:, None]).sum(axis=0) * 10000 + ct.sum(axis=0)
        cost = cost + np.where(sizes >= CREAL, big, 0)
        c = int(np.argmin(cost))
        colors[n] = c
        sizes[c] += 1
        np.add.at(cnt[:, c], t, 1)
    # refinement passes: re-greedy each node with warm counts (size-neutral
    # moves only happen when strictly better, tracking exact sizes)
    for _ in range(2):
        for n in proc:
            t = tgt_by_src[indptr[n]:indptr[n + 1]]
            c0 = colors[n]
            np.add.at(cnt[:, c0], t, -1)
            sizes[c0] -= 1
            ct = cnt[t, :]
            cost = ((ct >= quota[t][:, None]).sum(axis=0) * 10000
                    + (ct >= (quota[t][:, None] + 1)).sum(axis=0) * 100000
                    + ct.sum(axis=0))
            cost = cost + np.where(sizes >= CREAL, big, 0)
            c = int(np.argmin(cost))
            colors[n] = c
            sizes[c] += 1
            np.add.at(cnt[:, c], t, 1)
    return colors


def _host_prep(x, edge_index, w1, b1, c1, w2, b2, c2):
    src = np.asarray(edge_index[0], dtype=np.int64)
    tgt = np.asarray(edge_index[1], dtype=np.int64)
    x = np.asarray(x, dtype=np.float32)

    deg = np.bincount(tgt, minlength=N_NODES)
    colors = _color_sources(src, tgt, deg)

    # within each color: in-degree-sorted round-robin onto its two cores
    core_of_node = np.empty(N_NODES, dtype=np.int64)
    j_of_node = np.empty(N_NODES, dtype=np.int64)
    for c in range(NC_COL):
        nodes_c = np.where(colors == c)[0]
        nodes_c = nodes_c[np.argsort(-deg[nodes_c], kind="stable")]
        core_of_node[nodes_c] = 2 * c + (np.arange(len(nodes_c)) % 2)
        j_of_node[nodes_c] = np.arange(len(nodes_c)) // 2
    pos_of_node = core_of_node * J + j_of_node

    degs_kj = np.zeros((K, J), dtype=np.int64)
    degs_kj[core_of_node, j_of_node] = deg

    # per-(group, color) slot rectangle heights, shared across cores
    ek = core_of_node[tgt]
    ej = j_of_node[tgt]
    ecol = colors[src].astype(np.int64)
    gs_all = ej // P
    cnt_gc = np.zeros((K, G, P, NC_COL), dtype=np.int64)
    np.add.at(cnt_gc, (ek, gs_all, ej % P, ecol), 1)
    Dgr = np.maximum(cnt_gc.max(axis=(0, 2)), 1)      # [G, NC_COL]

    # pack consecutive groups into batches with common per-color heights
    batches = []        # dicts: ga, ng, Ds[4], slot_off (gt cols), idx_off
    icol_off = 0
    g = 0
    while g < G:
        ng = 1
        Ds = Dgr[g].copy()
        while g + ng < G and ng < NGMAX:
            nD = np.maximum(Ds, Dgr[g + ng])
            if (ng + 1) * int(nD.max()) > RCAP:
                break
            Ds = nD
            ng += 1
        batches.append({
            "ga": g, "ng": ng, "Ds": [int(d) for d in Ds],
            "icol": icol_off,
        })
        icol_off += 8 * ng * int(Ds.sum())   # int16 cols (= rows/16)
        g += ng
    TOTC = icol_off

    # index tensor [K][128, TOTC] int16 (wrap-16, replicated across 8 bands)
    idx_all = np.zeros((K, P, TOTC), dtype=np.int16)
    # edge slot positions: sort edges by (core, target j, color) to rank them
    key = ((ek * J) + ej) * NC_COL + ecol
    eorder = np.argsort(key, kind="stable")
    skey = key[eorder]
    _, counts = np.unique(skey, return_counts=True)
    run_starts = np.concatenate([[0], np.cumsum(counts)[:-1]])
    d_in_run = np.arange(len(skey)) - np.repeat(run_starts, counts)
    eks = skey // (J * NC_COL)
    ejs = (skey // NC_COL) % J
    ecs = skey % NC_COL
    egs = ejs // P
    eps = ejs % P
    rel = (pos_of_node[src[eorder]] - ecs * BLK).astype(np.int64)
    assert (rel >= 0).all() and (rel < BLK).all()

    ga_of_g = np.zeros(G, dtype=np.int64)
    for bi, bt in enumerate(batches):
        for gg in range(bt["ng"]):
            ga_of_g[bt["ga"] + gg] = bi
    # flat list position within the batch's idx block for each edge:
    # ranges laid out color-major: color R block has ng*Ds[R] columns of 128.
    bts = batches
    b_of_e = ga_of_g[egs]
    icol_b = np.array([bt["icol"] for bt in bts], dtype=np.int64)
    ng_b = np.array([bt["ng"] for bt in bts], dtype=np.int64)
    ga_b = np.array([bt["ga"] for bt in bts], dtype=np.int64)
    Ds_b = np.array([bt["Ds"] for bt in bts], dtype=np.int64)       # [B, 4]
    DsCum_b = np.concatenate([np.zeros((len(bts), 1), np.int64),
                              np.cumsum(Ds_b, axis=1)], axis=1)     # [B, 5]
    # column within gather dest for this edge's slot (color-major blocks):
    gg_e = egs - ga_b[b_of_e]
    cc = ng_b[b_of_e] * DsCum_b[b_of_e, ecs] + gg_e * Ds_b[b_of_e, ecs] + d_in_run
    jlist = cc * P + eps                   # position in the batch's flat list
    # band 0 (partitions 0-15): wrap-16 layout; pads -> PAD_REL (zero row)
    col16 = icol_b[b_of_e] + jlist // 16
    row16 = jlist % 16
    idx_all[:, :16, :] = PAD_REL
    idx_all[eks, row16, col16] = rel.astype(np.int16)
    # bands 1..7 are exact replicas of band 0
    for band in range(1, 8):
        idx_all[:, 16 * band:16 * (band + 1), :] = idx_all[:, :16, :]

    # per-core 1/deg  [K, P, G]  (0 for pad targets)
    with np.errstate(divide="ignore"):
        dr = 1.0 / np.maximum(degs_kj, 1).astype(np.float32)
    real = np.zeros((K, J), dtype=np.float32)
    real[:, :JREAL] = 1.0
    degrecip = (dr * np.where(real > 0, 1.0, 0.0)).reshape(K, G, P).transpose(0, 2, 1).copy()

    # pad-node mask [P, G] (same on every core)
    mask_j = (np.arange(J) < JREAL).astype(np.float32)
    mask_pg = mask_j.reshape(G, P).T.copy()

    # xT shards, bf16 [K][IN_F, J]
    xT = np.zeros((K, IN_F, J), dtype=ml_dtypes.bfloat16)
    for k in range(K):
        nodes_k = np.where(core_of_node == k)[0]
        xT[k][:, j_of_node[nodes_k]] = x[nodes_k].T.astype(ml_dtypes.bfloat16)

    # fused KAN weights
    A1 = (w1 + 0.1 * c1[:, :, 0]).astype(ml_dtypes.bfloat16)
    B1 = (0.1 * c1[:, :, 1]).astype(ml_dtypes.bfloat16)
    C1 = (0.1 * c1[:, :, 2]).astype(ml_dtypes.bfloat16)
    A2 = (w2 + 0.1 * c2[:, :, 0]).astype(ml_dtypes.bfloat16)
    B2 = (0.1 * c2[:, :, 1]).astype(ml_dtypes.bfloat16)
    C2 = (0.1 * c2[:, :, 2]).astype(ml_dtypes.bfloat16)
    b1b = np.tile(np.asarray(b1, np.float32)[None, :], (P, 1))
    b2b = np.tile(np.asarray(b2, np.float32)[None, :], (P, 1))
    ident = np.eye(P, dtype=np.float32)

    in_maps = []
    for k in range(K):
        in_maps.append({
            "xT": xT[k],
            "idx": idx_all[k],
            "degrecip": degrecip[k],
            "mask": mask_pg,
            "A1": A1, "B1": B1, "C1": C1,
            "A2": A2, "B2": B2, "C2": C2,
            "b1b": b1b, "b2b": b2b, "ident": ident,
        })
    meta = {"batches": batches, "TOTC": TOTC,
            "core_of_node": core_of_node, "j_of_node": j_of_node}
    return in_maps, meta


def build_program(batches, TOTC, dump=False):
    nc = bacc.Bacc("TRN2", target_bir_lowering=False, debug=False, num_devices=K,
                   dynamic_dma_scratch_size=32768, num_swdge_queues=4)

    xT = nc.dram_tensor("xT", [IN_F, J], BF16, kind="ExternalInput")
    idx = nc.dram_tensor("idx", [P, TOTC], I16, kind="ExternalInput")
    degrecip = nc.dram_tensor("degrecip", [P, G], F32, kind="ExternalInput")
    mask = nc.dram_tensor("mask", [P, G], F32, kind="ExternalInput")
    A1 = nc.dram_tensor("A1", [IN_F, HID_F], BF16, kind="ExternalInput")
    B1 = nc.dram_tensor("B1", [IN_F, HID_F], BF16, kind="ExternalInput")
    C1 = nc.dram_tensor("C1", [IN_F, HID_F], BF16, kind="ExternalInput")
    A2 = nc.dram_tensor("A2", [HID_F, OUT_F], BF16, kind="ExternalInput")
    B2 = nc.dram_tensor("B2", [HID_F, OUT_F], BF16, kind="ExternalInput")
    C2 = nc.dram_tensor("C2", [HID_F, OUT_F], BF16, kind="ExternalInput")
    b1b = nc.dram_tensor("b1b", [P, HID_F], F32, kind="ExternalInput")
    b2b = nc.dram_tensor("b2b", [P, OUT_F], F32, kind="ExternalInput")
    ident = nc.dram_tensor("ident", [P, P], F32, kind="ExternalInput")
    y = nc.dram_tensor("y", [J, OUT_F], F32, kind="ExternalOutput")
    if dump:
        h1o = nc.dram_tensor("h1o", [J, HID_F], BF16, kind="ExternalOutput")
        sno = nc.dram_tensor("sno", [J, HID_F], F32, kind="ExternalOutput")

    h1_in = nc.dram_tensor("h1_in", [J, HID_F], BF16, kind="Internal")
    h1_tbl = nc.dram_tensor("h1_tbl", [TBL, HID_F], BF16, kind="Internal",
                            addr_space="Shared")
    h2_in = nc.dram_tensor("h2_in", [J, OUT_F], F32, kind="Internal")
    h2_tbl = nc.dram_tensor("h2_tbl", [TBL, OUT_F], F32, kind="Internal",
                            addr_space="Shared")

    with tile.TileContext(nc) as tc:
        with (
            tc.tile_pool(name="consts", bufs=1) as cpool,
            tc.tile_pool(name="p1", bufs=2) as p1pool,
            tc.tile_pool(name="work", bufs=4) as wpool,
            tc.tile_pool(name="g1p", bufs=2) as g1pool,
            tc.tile_pool(name="idxp", bufs=2) as ipool,
            tc.tile_pool(name="psum", bufs=2, space="PSUM") as ppool,
            tc.tile_pool(name="psum2", bufs=3, space="PSUM") as ppool2,
        ):
            c_dr = cpool.tile([P, G], F32, tag="dr")
            nc.sync.dma_start(out=c_dr[:], in_=degrecip[:, :])
            c_mask = cpool.tile([P, G], F32, tag="mask")
            nc.sync.dma_start(out=c_mask[:], in_=mask[:, :])
            c_w1 = []
            for nm, t in (("A1", A1), ("B1", B1), ("C1", C1)):
                w = cpool.tile([IN_F, HID_F], BF16, tag=nm)
                nc.sync.dma_start(out=w[:], in_=t[:, :])
                c_w1.append(w)
            c_w2 = []
            for nm, t in (("A2", A2), ("B2", B2), ("C2", C2)):
                w = cpool.tile([HID_F, OUT_F], BF16, tag=nm)
                nc.sync.dma_start(out=w[:], in_=t[:, :])
                c_w2.append(w)
            c_b1 = cpool.tile([P, HID_F], F32, tag="b1b")
            nc.sync.dma_start(out=c_b1[:], in_=b1b[:, :])
            c_b2 = cpool.tile([P, OUT_F], F32, tag="b2b")
            nc.sync.dma_start(out=c_b2[:], in_=b2b[:, :])
            c_id = cpool.tile([P, P], F32, tag="ident")
            nc.sync.dma_start(out=c_id[:], in_=ident[:, :])

            # ---------------- phase 1: KAN layer 1 on the shard ----------------
            sc_p1, _ = nc.enter_named_scope("phase1", False)
            blocks = [(b * NB1, NB1) for b in range(96 // NB1)] + [(96, 1), (97, 1)]
            for (ga, nb) in blocks:
                w = nb * P
                xt = p1pool.tile([IN_F, NB1 * P], BF16, tag="xt")
                nc.sync.dma_start(out=xt[:, :w], in_=xT[:, ga * P:ga * P + w])
                x2 = p1pool.tile([IN_F, NB1 * P], BF16, tag="x2")
                nc.vector.tensor_tensor(out=x2[:, :w], in0=xt[:, :w], in1=xt[:, :w],
                                        op=mybir.AluOpType.mult)
                x3 = p1pool.tile([IN_F, NB1 * P], BF16, tag="x3")
                nc.vector.tensor_tensor(out=x3[:, :w], in0=x2[:, :w], in1=xt[:, :w],
                                        op=mybir.AluOpType.mult)
                ps = ppool.tile([P, NB1 * HID_F], F32, tag="k1")
                for gg in range(nb):
                    sl = slice(gg * P, (gg + 1) * P)
                    ol = slice(gg * HID_F, (gg + 1) * HID_F)
                    nc.tensor.matmul(out=ps[:, ol], lhsT=xt[:, sl], rhs=c_w1[0][:],
                                     start=True, stop=False)
                    nc.tensor.matmul(out=ps[:, ol], lhsT=x2[:, sl], rhs=c_w1[1][:],
                                     start=False, stop=False)
                    nc.tensor.matmul(out=ps[:, ol], lhsT=x3[:, sl], rhs=c_w1[2][:],
                                     start=False, stop=True)
                hb = p1pool.tile([P, NB1 * HID_F], F32, tag="hb")
                b1bc = c_b1[:, :].unsqueeze(1).to_broadcast([P, nb, HID_F])
                nc.vector.tensor_tensor(
                    out=hb[:, :nb * HID_F].rearrange("p (g f) -> p g f", g=nb),
                    in0=ps[:, :nb * HID_F].rearrange("p (g f) -> p g f", g=nb),
                    in1=b1bc, op=mybir.AluOpType.add)
                h1t = p1pool.tile([P, NB1 * HID_F], BF16, tag="h1t")
                if ga == 97:
                    nc.scalar.activation(out=h1t[:, :HID_F], in_=hb[:, :HID_F],
                                         func=mybir.ActivationFunctionType.Relu,
                                         scale=c_mask[:, 97:98])
                else:
                    nc.scalar.activation(out=h1t[:, :nb * HID_F],
                                         in_=hb[:, :nb * HID_F],
                                         func=mybir.ActivationFunctionType.Relu)
                nc.sync.dma_start(
                    out=h1_in[ga * P:(ga + nb) * P, :].rearrange(
                        "(g p) f -> p g f", p=P),
                    in_=h1t[:, :nb * HID_F].rearrange("p (g f) -> p g f", g=nb))
                if dump:
                    nc.sync.dma_start(
                        out=h1o[ga * P:(ga + nb) * P, :].rearrange(
                            "(g p) f -> p g f", p=P),
                        in_=h1t[:, :nb * HID_F].rearrange("p (g f) -> p g f", g=nb))
            nc.leave_named_scope("phase1", sc_p1, False)

            # ---------------- AllGather h1 ----------------
            sc_ag1, _ = nc.enter_named_scope("ag1", False)
            nc.gpsimd.collective_compute(
                "AllGather", mybir.AluOpType.bypass,
                replica_groups=[list(range(K))],
                ins=[h1_in[:, :]], outs=[h1_tbl[:, :]],
            )
            nc.leave_named_scope("ag1", sc_ag1, False)

            def gather_reduce(bt, table, F):
                """4 block gathers + strided tree reduce; returns [P, ng, F]
                view of the combined partial sums (in the color-0 block)."""
                ga, ng, Ds = bt["ga"], bt["ng"], bt["Ds"]
                dsum = sum(Ds)
                it = ipool.tile([P, 8 * 4 * RCAP], I16, tag="idx")
                ncols16 = 8 * ng * dsum
                nc.sync.dma_start(out=it[:, :ncols16],
                                  in_=idx[:, bt["icol"]:bt["icol"] + ncols16])
                # separate byte buffer per range block so the 4 queue-pairs
                # run concurrently and batch n+1 gathers overlap batch n
                # reduces; shared between phases via bitcast.
                views = []
                for R in range(NC_COL):
                    D = Ds[R]
                    nidx = P * ng * D
                    gt_raw = g1pool.tile([P, RCAP * HID_F], BF16, tag=f"g{R}")
                    gt = gt_raw[:, :] if table.dtype == BF16 else \
                        gt_raw[:, :].bitcast(F32)
                    nc.gpsimd.dma_gather(
                        out_ap=gt[:, :ng * D * F].rearrange(
                            "p (c f) -> p c f", c=ng * D),
                        in_ap=table[R * BLK:(R + 1) * BLK, :],
                        idxs_ap=it[:, 8 * ng * sum(Ds[:R]):
                                   8 * ng * sum(Ds[:R]) + nidx // 16],
                        num_idxs=nidx,
                        num_idxs_reg=nidx,
                        elem_size=F,
                        single_packet=False,
                        queue_num=R,
                    )
                    v = gt[:, :ng * D * F].rearrange("p (g x) -> p g x", g=ng)
                    cur = D
                    while cur > 1:
                        h = cur // 2
                        nc.vector.tensor_tensor(
                            out=v[:, :, :h * F], in0=v[:, :, :h * F],
                            in1=v[:, :, (cur - h) * F:cur * F],
                            op=mybir.AluOpType.add)
                        cur = cur - h
                    views.append(v[:, :, :F])
                acc = views[0]
                for R in range(1, NC_COL):
                    nc.vector.tensor_tensor(out=acc, in0=acc, in1=views[R],
                                            op=mybir.AluOpType.add)
                return acc

            # ---------------- phase 2: aggregate + KAN layer 2 ----------------
            sc_p2, _ = nc.enter_named_scope("phase2", False)
            for bt in batches:
                ga, ng = bt["ga"], bt["ng"]
                acc = gather_reduce(bt, h1_tbl, HID_F)
                sn = wpool.tile([P, NGMAX * HID_F], F32, tag="sn")
                drb = c_dr[:, ga:ga + ng].unsqueeze(2).to_broadcast([P, ng, HID_F])
                nc.vector.tensor_tensor(
                    out=sn[:, :ng * HID_F].rearrange("p (g f) -> p g f", g=ng),
                    in0=acc, in1=drb, op=mybir.AluOpType.mult)
                if dump:
                    nc.sync.dma_start(
                        out=sno[ga * P:(ga + ng) * P, :].rearrange(
                            "(g p) f -> p g f", p=P),
                        in_=sn[:, :ng * HID_F].rearrange("p (g f) -> p g f", g=ng))
                for gg in range(ng):
                    g = ga + gg
                    pt = ppool2.tile([P, P], F32, tag="tr")
                    nc.tensor.transpose(
                        out=pt[:], in_=sn[:, gg * HID_F:(gg + 1) * HID_F],
                        identity=c_id[:])
                    hT = wpool.tile([HID_F, P], BF16, tag="hT")
                    nc.scalar.copy(out=hT[:], in_=pt[:])
                    q2 = wpool.tile([HID_F, P], BF16, tag="q2")
                    nc.vector.tensor_tensor(out=q2[:], in0=hT[:], in1=hT[:],
                                            op=mybir.AluOpType.mult)
                    q3 = wpool.tile([HID_F, P], BF16, tag="q3")
                    nc.vector.tensor_tensor(out=q3[:], in0=q2[:], in1=hT[:],
                                            op=mybir.AluOpType.mult)
                    ps2 = ppool2.tile([P, OUT_F], F32, tag="k2")
                    nc.tensor.matmul(out=ps2[:], lhsT=hT[:], rhs=c_w2[0][:],
                                     start=True, stop=False)
                    nc.tensor.matmul(out=ps2[:], lhsT=q2[:], rhs=c_w2[1][:],
                                     start=False, stop=False)
                    nc.tensor.matmul(out=ps2[:], lhsT=q3[:], rhs=c_w2[2][:],
                                     start=False, stop=True)
                    hb2 = wpool.tile([P, OUT_F], F32, tag="hb2")
                    nc.vector.tensor_tensor(out=hb2[:], in0=ps2[:], in1=c_b2[:],
                                            op=mybir.AluOpType.add)
                    h2t = wpool.tile([P, OUT_F], F32, tag="h2t")
                    nc.scalar.activation(out=h2t[:], in_=hb2[:],
                                         func=mybir.ActivationFunctionType.Copy,
                                         scale=c_mask[:, g:g + 1])
                    nc.sync.dma_start(out=h2_in[g * P:(g + 1) * P, :], in_=h2t[:])
            nc.leave_named_scope("phase2", sc_p2, False)

            # ---------------- AllGather h2 ----------------
            sc_ag2, _ = nc.enter_named_scope("ag2", False)
            nc.gpsimd.collective_compute(
                "AllGather", mybir.AluOpType.bypass,
                replica_groups=[list(range(K))],
                ins=[h2_in[:, :]], outs=[h2_tbl[:, :]],
            )
            nc.leave_named_scope("ag2", sc_ag2, False)

            # ---------------- phase 3: aggregate + log_softmax ----------------
            sc_p3, _ = nc.enter_named_scope("phase3", False)
            tn_all = cpool.tile([P, G * OUT_F], F32, tag="tn_all")
            se_all = cpool.tile([P, G], F32, tag="se_all")
            for bt in batches:
                ga, ng = bt["ga"], bt["ng"]
                acc = gather_reduce(bt, h2_tbl, OUT_F)
                tnv = tn_all[:, ga * OUT_F:(ga + ng) * OUT_F].rearrange(
                    "p (g f) -> p g f", g=ng)
                drb = c_dr[:, ga:ga + ng].unsqueeze(2).to_broadcast([P, ng, OUT_F])
                nc.vector.tensor_tensor(out=tnv, in0=acc, in1=drb,
                                        op=mybir.AluOpType.mult)
                mxb = wpool.tile([P, NGMAX], F32, tag="mxb")
                nc.vector.tensor_reduce(out=mxb[:, :ng].unsqueeze(2), in_=tnv,
                                        axis=mybir.AxisListType.X,
                                        op=mybir.AluOpType.max)
                nc.vector.tensor_tensor(
                    out=tnv, in0=tnv,
                    in1=mxb[:, :ng].unsqueeze(2).to_broadcast([P, ng, OUT_F]),
                    op=mybir.AluOpType.subtract)
                et = wpool.tile([P, NGMAX * OUT_F], F32, tag="et")
                nc.scalar.activation(
                    out=et[:, :ng * OUT_F],
                    in_=tn_all[:, ga * OUT_F:(ga + ng) * OUT_F],
                    func=mybir.ActivationFunctionType.Exp)
                nc.vector.tensor_reduce(
                    out=se_all[:, ga:ga + ng].unsqueeze(2),
                    in_=et[:, :ng * OUT_F].rearrange("p (g f) -> p g f", g=ng),
                    axis=mybir.AxisListType.X, op=mybir.AluOpType.add)
            lse = cpool.tile([P, G], F32, tag="lse")
            nc.scalar.activation(out=lse[:], in_=se_all[:],
                                 func=mybir.ActivationFunctionType.Ln)
            for bt in batches:
                ga, ng = bt["ga"], bt["ng"]
                ot = wpool.tile([P, NGMAX * OUT_F], F32, tag="ot")
                nc.vector.tensor_tensor(
                    out=ot[:, :ng * OUT_F].rearrange("p (g f) -> p g f", g=ng),
                    in0=tn_all[:, ga * OUT_F:(ga + ng) * OUT_F].rearrange(
                        "p (g f) -> p g f", g=ng),
                    in1=lse[:, ga:ga + ng].unsqueeze(2).to_broadcast(
                        [P, ng, OUT_F]),
                    op=mybir.AluOpType.subtract)
                nc.sync.dma_start(
                    out=y[ga * P:(ga + ng) * P, :].rearrange(
                        "(g p) f -> p g f", p=P),
                    in_=ot[:, :ng * OUT_F].rearrange("p (g f) -> p g f", g=ng))
            nc.leave_named_scope("phase3", sc_p3, False)

    nc.compile()
    return nc


def kernel(x, edge_index, w1, b1, c1, w2, b2, c2):
    in_maps, meta = _host_prep(x, edge_index, w1, b1, c1, w2, b2, c2)
    nc = build_program(meta["batches"], meta["TOTC"])
    res = run_bass_kernel_spmd(nc, in_maps, core_ids=list(range(K)))
    out = np.empty((N_NODES, OUT_F), dtype=np.float32)
    core_of, j_of = meta["core_of_node"], meta["j_of_node"]
    for k in range(K):
        nodes_k = np.where(core_of == k)[0]
        out[nodes_k] = res.results[k]["y"][j_of[nodes_k]]
    return out
